# revision 1
# baseline (speedup 1.0000x reference)
"""AdaIN (segment mean/std + EMA of style stats) distributed over 8 TRN2 NeuronCores.

Strategy (data-parallel over rows):
  - content (1M, 64) and style (250K, 64) rows are sharded across 8 cores
    (padded per-core to a multiple of 128 rows; pad rows carry idx=16 which
    no one-hot column matches, so they contribute nothing).
  - pass 1 (per core): per 128-row block, build a one-hot (128,16) of the
    batch index with iota+compare, then one bf16 matmul per block
    accumulates [sum(x) | sum(x^2) | count] per segment into PSUM (16,129).
  - one 16.5KB AllReduce combines per-core partial sums for content+style.
  - stats math on every core (replicated, tiny): mean/std per segment, EMA
    across batch ids via a precomputed 16x16 lower-triangular matrix matmul,
    then per-segment coefficients a = g_std/c_std, b = g_mean - c_mean*a.
  - pass 2 (per core): per 128-row block, gather a/b rows per data row via
    (PE-transposed one-hot) @ [a|b] matmul into PSUM, then vector FMA
    out = x * a_g + b_g, DMA out.
"""

import os
import sys

import numpy as np

for _p in ("/opt/trn_rl_repo",):
    if _p not in sys.path and os.path.isdir(_p):
        sys.path.insert(0, _p)

from concourse import bacc, bass, bass_utils, masks, mybir, tile

F32 = mybir.dt.float32
BF16 = mybir.dt.bfloat16
I32 = mybir.dt.int32

N_CORES = 8
C = 64
B = 16
ALPHA = 0.1
EPS = 1e-8

# full problem sizes (hardcoded per problem spec)
NC_FULL = 1_000_000
NS_FULL = 250_000

# per-core padded rows (multiples of 128)
RC = 128 * 977  # 125056 ; 8*RC = 1000448 >= 1M
RS = 128 * 245  # 31360  ; 8*RS = 250880 >= 250K

BISECT = "full"  # debug: p1only | nocoll | nopass2 | full

SUP = 64  # blocks (of 128 rows) per super-tile
TG2 = 8   # blocks per pass-2 transpose->evict chunk (16x1024 bf16 = 1 PSUM bank)
GCH = 12  # blocks per pass-2 gather/FMA chunk (3 PSUM banks)


def _ema_lhsT() -> np.ndarray:
    """L[b, j] = weight of style-stats row j in globals used by batch b; return L^T."""
    L = np.zeros((B, B), np.float64)
    for b in range(B):
        L[b, 0] = (1.0 - ALPHA) ** b
        for j in range(1, b + 1):
            L[b, j] = ALPHA * (1.0 - ALPHA) ** (b - j)
    return np.ascontiguousarray(L.T).astype(np.float32)


def _chunks(total: int, step: int):
    t0 = 0
    while t0 < total:
        yield t0, min(step, total - t0)
        t0 += step


def build_nc(rc: int = RC, rs: int = RS, n_cores: int = N_CORES):
    """Build + compile the SPMD Bass graph. rc/rs are per-core row counts (mult of 128)."""
    ntc = rc // 128  # content blocks per core
    nts = rs // 128  # style blocks per core

    nc = bacc.Bacc(
        "TRN2", target_bir_lowering=False, debug=False, num_devices=n_cores
    )
    cf = nc.dram_tensor("cf", [rc, C], F32, kind="ExternalInput")
    ci = nc.dram_tensor("ci", [rc], I32, kind="ExternalInput")
    sf = nc.dram_tensor("sf", [rs, C], F32, kind="ExternalInput")
    si = nc.dram_tensor("si", [rs], I32, kind="ExternalInput")
    el = nc.dram_tensor("el", [B, B], F32, kind="ExternalInput")
    out = nc.dram_tensor("out", [rc, C], BF16, kind="ExternalOutput")

    # blocked views: partition p owns rows [p*nt, (p+1)*nt); block t = column t
    cf_v = cf.ap().rearrange("(p n) d -> p n d", p=128)
    ci_v = ci.ap().rearrange("(p n) -> p n", p=128)
    sf_v = sf.ap().rearrange("(p n) d -> p n d", p=128)
    si_v = si.ap().rearrange("(p n) -> p n", p=128)
    out_v = out.ap().rearrange("(p n) d -> p n d", p=128)

    with tile.TileContext(nc) as tc:
        with (
            tc.tile_pool(name="const", bufs=1) as constp,
            tc.tile_pool(name="dram", bufs=1, space="DRAM") as dramp,
        ):
            # --- constants ---
            # iota over 32 columns per block: cols 0:15 match real batch ids,
            # col 16 matches the pad sentinel (routed to zeroed coef rows),
            # cols 17:31 never match (zero pad for 32-aligned transposed strips)
            iota_rep = constp.tile([128, SUP, 2 * B], mybir.dt.int16)
            nc.gpsimd.iota(
                iota_rep[:], pattern=[[0, SUP], [1, 2 * B]], base=0,
                channel_multiplier=0,
            )
            ident = constp.tile([128, 128], BF16)
            masks.make_identity(nc, ident[:])
            el_sb = constp.tile([B, B], F32)
            nc.sync.dma_start(el_sb[:], el.ap())

            # --- resident index tiles ---
            ci_sb = constp.tile([128, ntc], I32)
            nc.sync.dma_start(ci_sb[:], ci_v)
            si_sb = constp.tile([128, nts], I32)
            nc.sync.dma_start(si_sb[:], si_v)

            # content one-hots (32-wide), built in pass 1, reused by pass 2
            oh32 = constp.tile([128, ntc, 2 * B], BF16)

            def pass1(feats_v, idx_sb, nt_total, ps, p1, p1in, oh_cache):
                n_done = 0
                for t0, nt in _chunks(nt_total, SUP):
                    ft = p1in.tile([128, SUP, C], F32, tag="p1f")
                    nc.sync.dma_start(ft[:, :nt, :], feats_v[:, t0 : t0 + nt, :])
                    rhs = p1.tile([128, SUP, 132], BF16, tag="p1r")
                    if oh_cache is not None:
                        oh = oh_cache[:, t0 : t0 + nt, :]
                    else:
                        oh_t = p1.tile([128, SUP, B], BF16, tag="p1oh")
                        oh = oh_t[:, :nt, :]
                    w = oh.shape[-1]
                    # producers at half-super-tile granularity; cast split
                    # DVE/ACT so neither starves the PE for long
                    for k, (h0, nh) in enumerate(_chunks(nt, SUP // 2)):
                        nc.scalar.activation(
                            rhs[:, h0 : h0 + nh, C : 2 * C],
                            ft[:, h0 : h0 + nh, :],
                            mybir.ActivationFunctionType.Square,
                        )
                        nc.vector.tensor_copy(
                            rhs[:, h0 : h0 + nh, 0:C], ft[:, h0 : h0 + nh, :]
                        )
                        nc.vector.memset(
                            rhs[:, h0 : h0 + nh, 2 * C : 2 * C + 1], 1.0
                        )
                        nc.vector.tensor_tensor(
                            oh[:, h0 : h0 + nh, :],
                            idx_sb[:, t0 + h0 : t0 + h0 + nh]
                            .unsqueeze(2)
                            .broadcast_to((128, nh, w)),
                            iota_rep[:, h0 : h0 + nh, 0:w],
                            mybir.AluOpType.is_equal,
                        )
                    for t in range(nt):
                        nc.tensor.matmul(
                            ps[:, 0 : 2 * C + 1],
                            oh[:, t, 0:B],
                            rhs[:, t, 0 : 2 * C + 1],
                            start=(n_done == 0),
                            stop=(n_done == nt_total - 1),
                        )
                        n_done += 1

            # --- stats math helper (tiny, replicated) ---
            def seg_stats(sums, ssq, cnt, mean_out, std_out):
                rc_ = constp.tile([B, 1], F32, tag="t1")
                nc.vector.reciprocal(rc_[:], cnt)
                nm1 = constp.tile([B, 1], F32, tag="t2")
                nc.vector.tensor_scalar_add(nm1[:], cnt, -1.0)
                rnm1 = constp.tile([B, 1], F32, tag="t3")
                nc.vector.reciprocal(rnm1[:], nm1[:])
                fac = constp.tile([B, 1], F32, tag="t4")
                nc.vector.tensor_tensor(fac[:], cnt, rnm1[:], mybir.AluOpType.mult)
                nc.vector.tensor_scalar_mul(mean_out, sums, rc_[:])
                ex2 = constp.tile([B, C], F32, tag="t5")
                nc.vector.tensor_scalar_mul(ex2[:], ssq, rc_[:])
                m2 = constp.tile([B, C], F32, tag="t6")
                nc.vector.tensor_tensor(m2[:], mean_out, mean_out, mybir.AluOpType.mult)
                var = constp.tile([B, C], F32, tag="t7")
                nc.vector.tensor_sub(var[:], ex2[:], m2[:])
                nc.vector.tensor_scalar_mul(var[:], var[:], fac[:])
                nc.vector.tensor_scalar_max(var[:], var[:], 0.0)
                nc.scalar.sqrt(std_out, var[:])
                nc.vector.tensor_scalar_add(std_out, std_out, EPS)

            def all_reduce(src_sb, dst_sb, tag):
                inb = dramp.tile([B, 2 * C + 1], F32, tag=f"in_{tag}")
                outb = dramp.tile([B, 2 * C + 1], F32, tag=f"out_{tag}")
                nc.sync.dma_start(inb[:], src_sb)
                if BISECT == "nocoll":
                    nc.sync.dma_start(outb[:], inb[:])
                else:
                    nc.gpsimd.collective_compute(
                        "AllReduce",
                        mybir.AluOpType.add,
                        replica_groups=[list(range(n_cores))],
                        ins=[inb.opt()],
                        outs=[outb.opt()],
                    )
                nc.sync.dma_start(dst_sb, outb[:])

            do_rest = BISECT != "p1only"
            gm_t = constp.tile([B, C], F32)
            gs_t = constp.tile([B, C], F32)
            # --- pass 1 + per-input stats necks ---
            with (
                tc.tile_pool(name="p1", bufs=4) as p1,
                tc.tile_pool(name="p1in", bufs=3) as p1in,
                tc.tile_pool(name="ps_stats", bufs=1, space="PSUM") as psp,
            ):
                ps_s = psp.tile([B, 2 * C + 1], F32)
                pass1(sf_v, si_sb, nts, ps_s, p1, p1in, None)
                ps_c = psp.tile([B, 2 * C + 1], F32)
                pass1(cf_v, ci_sb, ntc, ps_c, p1, p1in, oh32)
                stats_cs = constp.tile([B, 2 * (2 * C + 1)], F32)
                nc.scalar.copy(stats_cs[:, 0 : 2 * C + 1], ps_c[:, :])
                nc.scalar.copy(stats_cs[:, 2 * C + 1 :], ps_s[:, :])

            if not do_rest:
                nc.sync.dma_start(out.ap()[0:B, 0:C], stats_cs[:, 0:C])

            if do_rest:
              if True:
                g_cs = constp.tile([B, 2 * (2 * C + 1)], F32)
                inb = dramp.tile([B, 2 * (2 * C + 1)], F32)
                outb = dramp.tile([B, 2 * (2 * C + 1)], F32)
                nc.sync.dma_start(inb[:], stats_cs[:])
                if BISECT == "nocoll":
                    nc.sync.dma_start(outb[:], inb[:])
                else:
                    nc.gpsimd.collective_compute(
                        "AllReduce",
                        mybir.AluOpType.add,
                        replica_groups=[list(range(n_cores))],
                        ins=[inb.opt()],
                        outs=[outb.opt()],
                    )
                nc.sync.dma_start(g_cs[:], outb[:])
                g_c = g_cs[:, 0 : 2 * C + 1]
                g_s = g_cs[:, 2 * C + 1 :]
                s_stats = constp.tile([B, 2 * C], F32)  # [mean_s | std_s]
                seg_stats(
                    g_s[:, 0:C], g_s[:, C : 2 * C], g_s[:, 2 * C : 2 * C + 1],
                    s_stats[:, 0:C], s_stats[:, C : 2 * C],
                )
                with tc.tile_pool(name="ps_ema", bufs=1, space="PSUM") as psge:
                    g_ps = psge.tile([B, 2 * C], F32)
                    nc.tensor.matmul(
                        g_ps[:], el_sb[:], s_stats[:], start=True, stop=True
                    )
                    nc.vector.tensor_copy(gm_t[:], g_ps[:, 0:C])
                    nc.vector.tensor_copy(gs_t[:], g_ps[:, C : 2 * C])

                # content stats, shortened chain: a = g_std / sqrt(var_c)
                rc_ = constp.tile([B, 1], F32, tag="t1")
                nc.vector.reciprocal(rc_[:], g_c[:, 2 * C : 2 * C + 1])
                nm1 = constp.tile([B, 1], F32, tag="t2")
                nc.vector.tensor_scalar_add(nm1[:], g_c[:, 2 * C : 2 * C + 1], -1.0)
                rnm1 = constp.tile([B, 1], F32, tag="t3")
                nc.vector.reciprocal(rnm1[:], nm1[:])
                fac = constp.tile([B, 1], F32, tag="t4")
                nc.vector.tensor_tensor(
                    fac[:], g_c[:, 2 * C : 2 * C + 1], rnm1[:], mybir.AluOpType.mult
                )
                mean_c = constp.tile([B, C], F32)
                nc.vector.tensor_scalar_mul(mean_c[:], g_c[:, 0:C], rc_[:])
                ex2 = constp.tile([B, C], F32, tag="t5")
                nc.vector.tensor_scalar_mul(ex2[:], g_c[:, C : 2 * C], rc_[:])
                m2 = constp.tile([B, C], F32, tag="t6")
                nc.scalar.square(m2[:], mean_c[:])
                var = constp.tile([B, C], F32, tag="t7")
                nc.vector.tensor_sub(var[:], ex2[:], m2[:])
                nc.vector.tensor_scalar_mul(var[:], var[:], fac[:])
                std_c = constp.tile([B, C], F32)
                nc.scalar.sqrt(std_c[:], var[:])
                coef = constp.tile([B, 2 * C], BF16)  # [a | b]
                rstd = constp.tile([B, C], F32)
                nc.vector.reciprocal(rstd[:], std_c[:])
                a_t = constp.tile([B, C], F32)
                nc.vector.tensor_tensor(
                    a_t[:], gs_t[:], rstd[:], mybir.AluOpType.mult
                )
                tmp = constp.tile([B, C], F32)
                nc.vector.tensor_tensor(
                    tmp[:], mean_c[:], a_t[:], mybir.AluOpType.mult
                )
                b_t = constp.tile([B, C], F32)
                nc.vector.tensor_sub(b_t[:], gm_t[:], tmp[:])
                nc.vector.tensor_copy(coef[:, 0:C], a_t[:])
                nc.vector.tensor_copy(coef[:, C : 2 * C], b_t[:])

                # block-diag coef for 4-blocks-per-matmul gathers:
                # rows 32q+j (j<16) hold coef[j] at cols [128q, 128q+128);
                # rows 32q+16.. stay zero so pad-sentinel one-hot col 16
                # gathers zeros.
                coef_bd = constp.tile([128, 4 * 2 * C], BF16)
                nc.gpsimd.memset(coef_bd[:], 0.0)
                for q in range(4):
                    nc.sync.dma_start(
                        coef_bd[32 * q : 32 * q + B, 128 * q : 128 * (q + 1)],
                        coef[:],
                    )

                if BISECT != 'nopass2':
                  # --- pass 2 ---
                  with (
                    tc.tile_pool(name="p2", bufs=3) as p2,
                    tc.tile_pool(name="p2in", bufs=4) as p2in,
                    tc.tile_pool(name="ps_t", bufs=2, space="PSUM") as pst,
                    tc.tile_pool(name="ps_g", bufs=2, space="PSUM") as psg2,
                  ):
                    for t0, nt in _chunks(ntc, SUP):
                        f2 = p2in.tile([128, SUP, C], F32, tag="p2f")
                        nc.sync.dma_start(f2[:, :nt, :], cf_v[:, t0 : t0 + nt, :])
                        ot = p2.tile([128, SUP, C], BF16, tag="p2o")
                        for c0, nb in _chunks(nt, GCH):
                            g_ps2 = psg2.tile([128, GCH, 2 * C], F32, tag="gath")
                            ohT_sb = p2.tile([128, (GCH // 4) * 128], BF16, tag="p2ohT")
                            for g0, ng in _chunks(nb, 4):
                                ohT_ps = pst.tile([128, 128], BF16, tag="ohT")
                                nc.tensor.transpose(
                                    ohT_ps[0 : ng * 2 * B, :],
                                    oh32[:, t0 + c0 + g0 : t0 + c0 + g0 + ng, :],
                                    ident[:],
                                )
                                sb_sl = ohT_sb[:, (g0 // 4) * 128 : (g0 // 4) * 128 + 128]
                                nc.scalar.copy(
                                    sb_sl[0 : ng * 2 * B, :], ohT_ps[0 : ng * 2 * B, :]
                                )
                                nc.tensor.matmul(
                                    g_ps2[:, g0 : g0 + ng, :],
                                    sb_sl[0 : ng * 2 * B, :],
                                    coef_bd[0 : ng * 2 * B, 0 : ng * 2 * C],
                                    start=True,
                                    stop=True,
                                )
                            mt = p2.tile([128, GCH, C], F32, tag="p2m")
                            nc.vector.tensor_tensor(
                                mt[:, :nb, :],
                                f2[:, c0 : c0 + nb, :],
                                g_ps2[:, :nb, 0:C],
                                mybir.AluOpType.mult,
                            )
                            nc.vector.tensor_tensor(
                                ot[:, c0 : c0 + nb, :],
                                mt[:, :nb, :],
                                g_ps2[:, :nb, C : 2 * C],
                                mybir.AluOpType.add,
                            )
                        nc.sync.dma_start(out_v[:, t0 : t0 + nt, :], ot[:, :nt, :])

    nc.compile()
    return nc


_NC_CACHE = {}


def _get_nc(rc=RC, rs=RS, n_cores=N_CORES):
    key = (rc, rs, n_cores)
    if key not in _NC_CACHE:
        _NC_CACHE[key] = build_nc(rc, rs, n_cores)
    return _NC_CACHE[key]


def _pad_rows(a: np.ndarray, total: int, fill) -> np.ndarray:
    pad = total - a.shape[0]
    if pad == 0:
        return np.ascontiguousarray(a)
    pad_shape = (pad,) + a.shape[1:]
    return np.concatenate([a, np.full(pad_shape, fill, a.dtype)], axis=0)


def kernel(
    content_feats: np.ndarray,
    style_feats: np.ndarray,
    content_batch_indices: np.ndarray,
    style_batch_indices: np.ndarray,
    num_batches=B,
) -> np.ndarray:
    n_c = content_feats.shape[0]
    n_s = style_feats.shape[0]
    cf = _pad_rows(np.asarray(content_feats, np.float32), N_CORES * RC, 0.0)
    ci = _pad_rows(np.asarray(content_batch_indices, np.int32), N_CORES * RC, B)
    sf = _pad_rows(np.asarray(style_feats, np.float32), N_CORES * RS, 0.0)
    si = _pad_rows(np.asarray(style_batch_indices, np.int32), N_CORES * RS, B)
    el = _ema_lhsT()

    nc = _get_nc()
    in_maps = [
        {
            "cf": np.ascontiguousarray(cf[k * RC : (k + 1) * RC]),
            "ci": np.ascontiguousarray(ci[k * RC : (k + 1) * RC]),
            "sf": np.ascontiguousarray(sf[k * RS : (k + 1) * RS]),
            "si": np.ascontiguousarray(si[k * RS : (k + 1) * RS]),
            "el": el,
        }
        for k in range(N_CORES)
    ]
    res = bass_utils.run_bass_kernel_spmd(nc, in_maps, core_ids=list(range(N_CORES)))
    out = np.concatenate(
        [np.asarray(res.results[k]["out"]) for k in range(N_CORES)], axis=0
    )
    return np.ascontiguousarray(out[:n_c]).astype(np.float32)



# revision 7
# speedup vs baseline: 1.4970x; 1.4970x over previous
"""AdaIN (segment mean/std + EMA of style stats) distributed over 8 TRN2 NeuronCores.

v3 strategy — host-side segment bucketing + transposed layout:
  - The host deals each segment's rows evenly across the 8 cores into
    fixed-capacity buckets (content: CAP rows per (core, segment); style:
    SCAP), padding with zero rows. Pad rows contribute nothing to sums, and
    exact per-segment counts are computed on the host.
  - Data ships TRANSPOSED: partition p = half*64 + channel, free dim = rows.
    Each segment occupies a fixed, compile-time-constant column range, so the
    SPMD instruction stream is static.
  - pass 1 (per core): per segment, DMA the f32 tile, cast to a persistent
    bf16 SBUF cache (fused with accum_out => per-channel sum(x)), and an ACT
    Square pass with accum_out => sum(x^2). Style is reduced straight from
    the staged f32 tiles (no cache).
  - one 32KB AllReduce combines per-core partial sums (the module's own
    dist behavior); a tiny f32 matmul folds the two row-halves together.
  - stats math replicated on every core in [channel, segment] orientation:
    mean/std via host-provided 1/n and n/(n-1), the EMA across batch ids as a
    single tensor_tensor_scan along the segment axis, then a = g_std/c_std,
    b = g_mean - c_mean*a per (channel, segment).
  - pass 2 (per core): per segment, out = x*a + b where a,b are per-partition
    scalars -- one fused op per engine-span on DVE (tensor_scalar), ACT
    (activation Identity w/ scale+bias) and GPSIMD (tensor_scalar), from the
    bf16 cache; bf16 out DMA'd to HBM. Host undoes the permutation.
"""

import os
import sys

import numpy as np

for _p in ("/opt/trn_rl_repo",):
    if _p not in sys.path and os.path.isdir(_p):
        sys.path.insert(0, _p)

from concourse import bacc, bass, bass_utils, mybir, tile

F32 = mybir.dt.float32
BF16 = mybir.dt.bfloat16

N_CORES = 8
C = 64
B = 16
ALPHA = 0.1
EPS = 1e-8

# per-(core, segment) bucket capacities (rows; must be even)
CAP = 7936    # content: expected ~7813 +- 30 after even dealing
SCAP = 1984   # style:   expected ~1953 +- 15

# pass-2 engine column split of each CAP//2-wide segment tile
P2_DVE = 1600
P2_ACT = 1600

BISECT = "full"


def _chunks(total: int, step: int):
    t0 = 0
    while t0 < total:
        yield t0, min(step, total - t0)
        t0 += step


def build_nc(cap: int = CAP, scap: int = SCAP, n_cores: int = N_CORES):
    cs = cap // 2     # content cols per segment
    ss = scap // 2    # style cols per segment
    CC = B * cs       # content cols per core
    SC = B * ss

    nc = bacc.Bacc(
        "TRN2", target_bir_lowering=False, debug=False, num_devices=n_cores
    )
    xin = nc.dram_tensor("xin", [128, CC], F32, kind="ExternalInput")
    sin = nc.dram_tensor("sin", [128, SC], F32, kind="ExternalInput")
    hc = nc.dram_tensor("hc", [C, 4 * B], F32, kind="ExternalInput")
    pairp = nc.dram_tensor("pairp", [128, C], F32, kind="ExternalInput")
    outb = nc.dram_tensor("outb", [128, CC], BF16, kind="ExternalOutput")

    ID = mybir.ActivationFunctionType.Identity
    SQ = mybir.ActivationFunctionType.Square

    with tile.TileContext(nc) as tc:
        with (
            tc.tile_pool(name="const", bufs=1) as constp,
            tc.tile_pool(name="dram", bufs=1, space="DRAM") as dramp,
        ):
            hc_sb = constp.tile([C, 4 * B], F32)
            nc.sync.dma_start(hc_sb[:], hc.ap())
            pairp_sb = constp.tile([128, C], F32)
            nc.sync.dma_start(pairp_sb[:], pairp.ap())

            xc = constp.tile([128, CC], BF16)       # content cache
            R = constp.tile([128, 4 * B], F32)      # [sx_c|sx2_c|sx_s|sx2_s]
            Rg = constp.tile([128, 4 * B], F32)     # after AllReduce

            # ---------------- pass 1 ----------------
            with (
                tc.tile_pool(name="p1s", bufs=2) as p1s,
                tc.tile_pool(name="p1c", bufs=2) as p1c,
                tc.tile_pool(name="p1d", bufs=2) as p1d,
            ):
                for s in range(B):
                    st = p1s.tile([128, ss], F32, tag="st")
                    nc.sync.dma_start(st[:], sin.ap()[:, s * ss : (s + 1) * ss])
                    d1 = p1d.tile([128, ss], BF16, tag="sd1")
                    nc.scalar.activation(
                        d1[:], st[:], SQ,
                        accum_out=R[:, 3 * B + s : 3 * B + s + 1],
                    )
                    d2 = p1d.tile([128, ss], BF16, tag="sd2")
                    nc.vector.tensor_scalar(
                        d2[:], st[:], 1.0, 0.0, mybir.AluOpType.mult,
                        mybir.AluOpType.add,
                        accum_out=R[:, 2 * B + s : 2 * B + s + 1],
                    )
                for s in range(B):
                    ct = p1c.tile([128, cs], F32, tag="ct")
                    nc.sync.dma_start(ct[:], xin.ap()[:, s * cs : (s + 1) * cs])
                    nc.vector.tensor_scalar(
                        xc[:, s * cs : (s + 1) * cs], ct[:], 1.0, 0.0,
                        mybir.AluOpType.mult, mybir.AluOpType.add,
                        accum_out=R[:, s : s + 1],
                    )
                    d3 = p1d.tile([128, cs], BF16, tag="cd")
                    nc.scalar.activation(
                        d3[:], xc[:, s * cs : (s + 1) * cs], SQ,
                        accum_out=R[:, B + s : B + s + 1],
                    )

            # ---------------- AllReduce ----------------
            inb = dramp.tile([128, 4 * B], F32)
            outb_d = dramp.tile([128, 4 * B], F32)
            nc.sync.dma_start(inb[:], R[:])
            if BISECT == "nocoll":
                nc.sync.dma_start(outb_d[:], inb[:])
            else:
                nc.gpsimd.collective_compute(
                    "AllReduce",
                    mybir.AluOpType.add,
                    replica_groups=[list(range(n_cores))],
                    ins=[inb.opt()],
                    outs=[outb_d.opt()],
                )
            nc.sync.dma_start(Rg[:], outb_d[:])

            # fold the two row-halves: S[c, j] = Rg[c, j] + Rg[64+c, j]
            S = constp.tile([C, 4 * B], F32)
            with tc.tile_pool(name="ps_fold", bufs=1, space="PSUM") as psf:
                ps = psf.tile([C, 4 * B], F32)
                nc.tensor.matmul(ps[:], pairp_sb[:], Rg[:], start=True, stop=True)
                nc.vector.tensor_copy(S[:], ps[:])

            # ---------------- stats math ([channel, segment]) --------------
            rn_c, fac_c = hc_sb[:, 0:B], hc_sb[:, B : 2 * B]
            rn_s, fac_s = hc_sb[:, 2 * B : 3 * B], hc_sb[:, 3 * B : 4 * B]

            def seg_stats(sx, sx2, rn, fac, mean_out, std_out):
                nc.vector.tensor_tensor(mean_out, sx, rn, mybir.AluOpType.mult)
                ex2 = constp.tile([C, B], F32, tag="ts_ex2")
                nc.vector.tensor_tensor(ex2[:], sx2, rn, mybir.AluOpType.mult)
                m2 = constp.tile([C, B], F32, tag="ts_m2")
                nc.scalar.square(m2[:], mean_out)
                var = constp.tile([C, B], F32, tag="ts_var")
                nc.vector.tensor_sub(var[:], ex2[:], m2[:])
                nc.vector.tensor_tensor(var[:], var[:], fac, mybir.AluOpType.mult)
                nc.vector.tensor_scalar_max(var[:], var[:], 0.0)
                nc.scalar.sqrt(std_out, var[:])
                nc.vector.tensor_scalar_add(std_out, std_out, EPS)

            mean_c = constp.tile([C, B], F32)
            std_c = constp.tile([C, B], F32)
            seg_stats(S[:, 0:B], S[:, B : 2 * B], rn_c, fac_c, mean_c[:], std_c[:])
            mean_s = constp.tile([C, B], F32)
            std_s = constp.tile([C, B], F32)
            seg_stats(S[:, 2 * B : 3 * B], S[:, 3 * B :], rn_s, fac_s,
                      mean_s[:], std_s[:])

            # EMA along segments as one scan: g_j = 0.9*g_{j-1} + w_j*s_j,
            # w_0 = 1 (globals start as batch 0's style stats), w_j = 0.1
            sm = constp.tile([128, B], F32)  # [mean_s ; std_s] stacked
            nc.sync.dma_start(sm[0:C, :], mean_s[:])
            nc.sync.dma_start(sm[C:128, :], std_s[:])
            smw = constp.tile([128, B], F32)
            nc.vector.tensor_scalar_mul(smw[:], sm[:], ALPHA)
            nc.vector.tensor_copy(smw[:, 0:1], sm[:, 0:1])
            c09 = constp.tile([128, B], F32)
            nc.vector.memset(c09[:], 1.0 - ALPHA)
            g = constp.tile([128, B], F32)
            nc.vector.tensor_tensor_scan(
                g[:], c09[:], smw[:], 0.0,
                mybir.AluOpType.mult, mybir.AluOpType.add,
            )

            # a = g_std / std_c ; b = g_mean - mean_c * a
            gs2 = constp.tile([C, B], F32)
            nc.sync.dma_start(gs2[:], g[C:128, :])
            rstd = constp.tile([C, B], F32)
            nc.vector.reciprocal(rstd[:], std_c[:])
            a_t = constp.tile([C, B], F32)
            nc.vector.tensor_tensor(a_t[:], gs2[:], rstd[:], mybir.AluOpType.mult)
            amc = constp.tile([C, B], F32)
            nc.vector.tensor_tensor(amc[:], mean_c[:], a_t[:], mybir.AluOpType.mult)
            b_t = constp.tile([C, B], F32)
            nc.vector.tensor_sub(b_t[:], g[0:C, :], amc[:])

            coefA = constp.tile([128, B], F32)
            coefB = constp.tile([128, B], F32)
            nc.sync.dma_start(coefA[0:C, :], a_t[:])
            nc.sync.dma_start(coefA[C:128, :], a_t[:])
            nc.sync.dma_start(coefB[0:C, :], b_t[:])
            nc.sync.dma_start(coefB[C:128, :], b_t[:])

            if BISECT == "nopass2":
                nc.sync.dma_start(outb.ap()[0:C, 0:B], coefA[0:C, :])
                nc.compile()
                return nc

            # ---------------- pass 2 ----------------
            d0, d1_ = P2_DVE, P2_DVE + P2_ACT
            with tc.tile_pool(name="p2o", bufs=3) as p2o:
                for s in range(B):
                    base = s * cs
                    ot = p2o.tile([128, cs], BF16, tag="ot")
                    sa = coefA[:, s : s + 1]
                    sb = coefB[:, s : s + 1]
                    nc.vector.tensor_scalar(
                        ot[:, 0:d0], xc[:, base : base + d0], sa, sb,
                        mybir.AluOpType.mult, mybir.AluOpType.add,
                    )
                    nc.scalar.activation(
                        ot[:, d0:d1_], xc[:, base + d0 : base + d1_], ID,
                        bias=sb, scale=sa,
                    )
                    nc.gpsimd.tensor_scalar(
                        ot[:, d1_:cs], xc[:, base + d1_ : base + cs], sa, sb,
                        mybir.AluOpType.mult, mybir.AluOpType.add,
                    )
                    nc.sync.dma_start(outb.ap()[:, base : base + cs], ot[:])

    nc.compile()
    return nc


_NC_CACHE = {}


def _get_nc(cap=CAP, scap=SCAP, n_cores=N_CORES):
    key = (cap, scap, n_cores)
    if key not in _NC_CACHE:
        _NC_CACHE[key] = build_nc(cap, scap, n_cores)
    return _NC_CACHE[key]


def _deal(idx: np.ndarray, cap: int, n_cores: int):
    """Deal each segment's rows evenly across cores into cap-sized buckets.

    Returns G[(core, seg, cap)] int64 row ids, with N (== len(idx)) marking
    pad slots, and the exact per-segment counts.
    """
    n = len(idx)
    order = np.argsort(idx, kind="stable")
    counts = np.bincount(idx, minlength=B)[:B]
    G = np.full((n_cores, B, cap), n, dtype=np.int64)
    off = 0
    for s in range(B):
        rows_s = order[off : off + counts[s]]
        off += counts[s]
        splits = (np.arange(n_cores + 1) * counts[s]) // n_cores
        for k in range(n_cores):
            ck = rows_s[splits[k] : splits[k + 1]]
            G[k, s, : len(ck)] = ck
    return G, counts


def _to_device_layout(feats: np.ndarray, G: np.ndarray, cap: int):
    """(N, 64) f32 + bucket map -> per-core [128, B*cap//2] f32 arrays."""
    n = feats.shape[0]
    fz = np.concatenate([feats, np.zeros((1, C), np.float32)], axis=0)
    res = []
    for k in range(G.shape[0]):
        Xk = fz[G[k].reshape(-1)]                      # (B*cap, 64)
        B4 = Xk.reshape(B, 2, cap // 2, C)             # (s, h, r, c)
        res.append(
            np.ascontiguousarray(
                B4.transpose(1, 3, 0, 2).reshape(128, B * (cap // 2))
            )
        )
    return res


def _host_inputs(content_feats, style_feats, content_batch_indices,
                 style_batch_indices, cap, scap):
    cfe = np.asarray(content_feats, np.float32)
    sfe = np.asarray(style_feats, np.float32)
    cidx = np.asarray(content_batch_indices, np.int64)
    sidx = np.asarray(style_batch_indices, np.int64)

    Gc, cnt_c = _deal(cidx, cap, N_CORES)
    Gs, cnt_s = _deal(sidx, scap, N_CORES)
    xins = _to_device_layout(cfe, Gc, cap)
    sins = _to_device_layout(sfe, Gs, scap)

    nc_ = np.maximum(cnt_c.astype(np.float64), 1.0)
    ns_ = np.maximum(cnt_s.astype(np.float64), 1.0)
    hrow = np.concatenate(
        [1.0 / nc_, nc_ / np.maximum(nc_ - 1.0, 1.0),
         1.0 / ns_, ns_ / np.maximum(ns_ - 1.0, 1.0)]
    ).astype(np.float32)
    hc = np.ascontiguousarray(np.tile(hrow[None, :], (C, 1)))
    p = np.arange(128)
    pairp = (p[:, None] % C == np.arange(C)[None, :]).astype(np.float32)

    in_maps = [
        {"xin": xins[k], "sin": sins[k], "hc": hc, "pairp": pairp}
        for k in range(N_CORES)
    ]
    return in_maps, Gc


def _assemble_output(results, Gc, cap, n_rows):
    out = np.zeros((n_rows, C), np.float32)
    for k in range(N_CORES):
        O = np.asarray(results[k]["outb"]).astype(np.float32)
        rows = (
            O.reshape(2, C, B, cap // 2)
            .transpose(2, 0, 3, 1)
            .reshape(B * cap, C)
        )
        gk = Gc[k].reshape(-1)
        mask = gk < n_rows
        out[gk[mask]] = rows[mask]
    return out


def _pick_caps(content_batch_indices, style_batch_indices):
    cidx = np.asarray(content_batch_indices, np.int64)
    sidx = np.asarray(style_batch_indices, np.int64)
    need_c = int(np.ceil(np.bincount(cidx, minlength=B)[:B].max() / N_CORES))
    need_s = int(np.ceil(np.bincount(sidx, minlength=B)[:B].max() / N_CORES))

    def rnd(x):
        return ((x + 63) // 64) * 64

    return max(CAP, rnd(need_c)), max(SCAP, rnd(need_s))


def kernel(
    content_feats: np.ndarray,
    style_feats: np.ndarray,
    content_batch_indices: np.ndarray,
    style_batch_indices: np.ndarray,
    num_batches=B,
) -> np.ndarray:
    n_c = content_feats.shape[0]
    cap, scap = _pick_caps(content_batch_indices, style_batch_indices)
    in_maps, Gc = _host_inputs(
        content_feats, style_feats, content_batch_indices,
        style_batch_indices, cap, scap,
    )
    nc = _get_nc(cap, scap)
    res = bass_utils.run_bass_kernel_spmd(nc, in_maps, core_ids=list(range(N_CORES)))
    return _assemble_output(res.results, Gc, cap, n_c)


# revision 12
# speedup vs baseline: 1.5502x; 1.0356x over previous
"""AdaIN (segment mean/std + EMA of style stats) distributed over 8 TRN2 NeuronCores.

v3 strategy — host-side segment bucketing + transposed layout:
  - The host deals each segment's rows evenly across the 8 cores into
    fixed-capacity buckets (content: CAP rows per (core, segment); style:
    SCAP), padding with zero rows. Pad rows contribute nothing to sums, and
    exact per-segment counts are computed on the host.
  - Data ships TRANSPOSED: partition p = half*64 + channel, free dim = rows.
    Each segment occupies a fixed, compile-time-constant column range, so the
    SPMD instruction stream is static.
  - pass 1 (per core): per segment, DMA the f32 tile, cast to a persistent
    bf16 SBUF cache (fused with accum_out => per-channel sum(x)), and an ACT
    Square pass with accum_out => sum(x^2). Style is reduced straight from
    the staged f32 tiles (no cache).
  - one 32KB AllReduce combines per-core partial sums (the module's own
    dist behavior); a tiny f32 matmul folds the two row-halves together.
  - stats math replicated on every core in [channel, segment] orientation:
    mean/std via host-provided 1/n and n/(n-1), the EMA across batch ids as a
    single tensor_tensor_scan along the segment axis, then a = g_std/c_std,
    b = g_mean - c_mean*a per (channel, segment).
  - pass 2 (per core): per segment, out = x*a + b where a,b are per-partition
    scalars -- one fused op per engine-span on DVE (tensor_scalar), ACT
    (activation Identity w/ scale+bias) and GPSIMD (tensor_scalar), from the
    bf16 cache; bf16 out DMA'd to HBM. Host undoes the permutation.
"""

import os
import sys

import numpy as np

for _p in ("/opt/trn_rl_repo",):
    if _p not in sys.path and os.path.isdir(_p):
        sys.path.insert(0, _p)

from concourse import bacc, bass, bass_utils, mybir, tile

F32 = mybir.dt.float32
BF16 = mybir.dt.bfloat16

N_CORES = 8
C = 64
B = 16
ALPHA = 0.1
EPS = 1e-8

# per-(core, segment) bucket capacities (rows; must be even)
CAP = 7936    # content: expected ~7813 +- 30 after even dealing
SCAP = 1984   # style:   expected ~1953 +- 15

# pass-2 engine column split of each CAP//2-wide segment tile
P2_DVE = 1600
P2_ACT = 1600

BISECT = "full"


def _chunks(total: int, step: int):
    t0 = 0
    while t0 < total:
        yield t0, min(step, total - t0)
        t0 += step


def build_nc(cap: int = CAP, scap: int = SCAP, n_cores: int = N_CORES):
    cs = cap // 2     # content cols per segment
    ss = scap // 2    # style cols per segment
    CC = B * cs       # content cols per core
    SC = B * ss

    nc = bacc.Bacc(
        "TRN2", target_bir_lowering=False, debug=False, num_devices=n_cores
    )
    xin = nc.dram_tensor("xin", [128, CC], F32, kind="ExternalInput")
    sin = nc.dram_tensor("sin", [128, SC], F32, kind="ExternalInput")
    hc = nc.dram_tensor("hc", [128, 4 * B], F32, kind="ExternalInput")
    pairp = nc.dram_tensor("pairp", [128, 128], F32, kind="ExternalInput")
    mask_a = nc.dram_tensor("mka", [128, 2], F32, kind="ExternalInput")
    outb = nc.dram_tensor("outb", [128, CC], BF16, kind="ExternalOutput")

    ID = mybir.ActivationFunctionType.Identity
    SQ = mybir.ActivationFunctionType.Square

    with tile.TileContext(nc) as tc:
        with (
            tc.tile_pool(name="const", bufs=1) as constp,
            tc.tile_pool(name="dram", bufs=1, space="DRAM") as dramp,
        ):
            hc_sb = constp.tile([128, 4 * B], F32)
            nc.sync.dma_start(hc_sb[:], hc.ap())
            pairp_sb = constp.tile([128, 128], F32)
            nc.sync.dma_start(pairp_sb[:], pairp.ap())
            mka = constp.tile([128, 2], F32)   # col0: 1 on 0:64; col1: 1 on 64:128
            nc.sync.dma_start(mka[:], mask_a.ap())

            xc = constp.tile([128, CC], BF16)        # content cache
            Rv = constp.tile([128, 2 * B], F32)      # DVE accums [sx_c|sx_s]
            Ra = constp.tile([128, 2 * B], F32)      # ACT accums [sx2_c|sx2_s]
            Rg = constp.tile([128, 4 * B], F32)      # after AllReduce

            # ---------------- pass 1 ----------------
            with (
                tc.tile_pool(name="p1s", bufs=2) as p1s,
                tc.tile_pool(name="p1c", bufs=3) as p1c,
                tc.tile_pool(name="p1d", bufs=2) as p1d,
            ):
                for s in range(B):
                    st = p1s.tile([128, ss], F32, tag="st")
                    h = ss // 2
                    nc.sync.dma_start(st[:, 0:h], sin.ap()[:, s * ss : s * ss + h])
                    nc.sync.dma_start(
                        st[:, h:ss], sin.ap()[:, s * ss + h : (s + 1) * ss]
                    )
                    d1 = p1d.tile([128, ss], BF16, tag="sd1")
                    nc.scalar.activation(
                        d1[:], st[:], SQ,
                        accum_out=Ra[:, B + s : B + s + 1],
                    )
                    d2 = p1d.tile([128, ss], BF16, tag="sd2")
                    nc.vector.tensor_scalar(
                        d2[:], st[:], 1.0, 0.0, mybir.AluOpType.mult,
                        mybir.AluOpType.add,
                        accum_out=Rv[:, B + s : B + s + 1],
                    )
                for s in range(B):
                    ct = p1c.tile([128, cs], F32, tag="ct")
                    h = cs // 2
                    nc.sync.dma_start(ct[:, 0:h], xin.ap()[:, s * cs : s * cs + h])
                    nc.sync.dma_start(
                        ct[:, h:cs], xin.ap()[:, s * cs + h : (s + 1) * cs]
                    )
                    nc.vector.tensor_scalar(
                        xc[:, s * cs : (s + 1) * cs], ct[:], 1.0, 0.0,
                        mybir.AluOpType.mult, mybir.AluOpType.add,
                        accum_out=Rv[:, s : s + 1],
                    )
                    d3 = p1d.tile([128, cs], BF16, tag="cd")
                    nc.scalar.activation(
                        d3[:], xc[:, s * cs : (s + 1) * cs], SQ,
                        accum_out=Ra[:, s : s + 1],
                    )

            # ---------------- AllReduce ----------------
            # dram layout: [sx_c | sx_s | sx2_c | sx2_s]
            inb = dramp.tile([128, 4 * B], F32)
            outb_d = dramp.tile([128, 4 * B], F32)
            nc.sync.dma_start(inb[:, 0 : 2 * B], Rv[:])
            nc.sync.dma_start(inb[:, 2 * B : 4 * B], Ra[:])
            if BISECT == "nocoll":
                nc.sync.dma_start(outb_d[:], inb[:])
            else:
                nc.gpsimd.collective_compute(
                    "AllReduce",
                    mybir.AluOpType.add,
                    replica_groups=[list(range(n_cores))],
                    ins=[inb.opt()],
                    outs=[outb_d.opt()],
                )
            nc.sync.dma_start(Rg[:], outb_d[:])

            # fold the two row-halves, replicating the result onto both
            # halves: S[p, j] = Rg[p%64, j] + Rg[64 + p%64, j]
            S = constp.tile([128, 4 * B], F32)
            with tc.tile_pool(name="ps_fold", bufs=1, space="PSUM") as psf:
                ps = psf.tile([128, 4 * B], F32)
                nc.tensor.matmul(ps[:], pairp_sb[:], Rg[:], start=True, stop=True)
                nc.vector.tensor_copy(S[:], ps[:])

            # ------------- stats math ([channel, segment], replicated) ------
            rn_c, fac_c = hc_sb[:, 0:B], hc_sb[:, B : 2 * B]
            rn_s, fac_s = hc_sb[:, 2 * B : 3 * B], hc_sb[:, 3 * B : 4 * B]

            def seg_stats(sx, sx2, rn, fac, mean_out, std_out):
                nc.vector.tensor_tensor(mean_out, sx, rn, mybir.AluOpType.mult)
                ex2 = constp.tile([128, B], F32, tag="ts_ex2")
                nc.vector.tensor_tensor(ex2[:], sx2, rn, mybir.AluOpType.mult)
                m2 = constp.tile([128, B], F32, tag="ts_m2")
                nc.scalar.square(m2[:], mean_out)
                var = constp.tile([128, B], F32, tag="ts_var")
                nc.vector.tensor_sub(var[:], ex2[:], m2[:])
                nc.vector.tensor_tensor(var[:], var[:], fac, mybir.AluOpType.mult)
                nc.vector.tensor_scalar_max(var[:], var[:], 0.0)
                nc.scalar.sqrt(std_out, var[:])
                nc.vector.tensor_scalar_add(std_out, std_out, EPS)

            mean_c = constp.tile([128, B], F32)
            std_c = constp.tile([128, B], F32)
            seg_stats(S[:, 0:B], S[:, 2 * B : 3 * B], rn_c, fac_c,
                      mean_c[:], std_c[:])
            mean_s = constp.tile([128, B], F32)
            std_s = constp.tile([128, B], F32)
            seg_stats(S[:, B : 2 * B], S[:, 3 * B :], rn_s, fac_s,
                      mean_s[:], std_s[:])

            # EMA along segments as one scan: g_j = 0.9*g_{j-1} + w_j*s_j,
            # w_0 = 1 (globals start as batch 0's style stats), w_j = 0.1.
            # Scan input: mean_s on partitions 0:64, std_s on 64:128 (both are
            # replicated, so mask-merge instead of partition moves).
            tmp_sb = constp.tile([128, B], F32)
            nc.vector.tensor_scalar_mul(tmp_sb[:], std_s[:], mka[:, 1:2])
            smw = constp.tile([128, B], F32)
            nc.vector.scalar_tensor_tensor(
                smw[:], mean_s[:], mka[:, 0:1], tmp_sb[:],
                mybir.AluOpType.mult, mybir.AluOpType.add,
            )
            # scale by ALPHA except column 0
            smk = constp.tile([128, B], F32)
            nc.vector.tensor_scalar_mul(smk[:], smw[:], ALPHA)
            nc.vector.tensor_copy(smk[:, 0:1], smw[:, 0:1])
            c09 = constp.tile([128, B], F32)
            nc.vector.memset(c09[:], 1.0 - ALPHA)
            g = constp.tile([128, B], F32)
            nc.vector.tensor_tensor_scan(
                g[:], c09[:], smk[:], 0.0,
                mybir.AluOpType.mult, mybir.AluOpType.add,
            )

            # a = g_std/std_c (valid on partitions 64:128);
            # replicate onto both halves, then b = g_mean - mean_c*a.
            rstd = constp.tile([128, B], F32)
            nc.vector.reciprocal(rstd[:], std_c[:])
            ag = constp.tile([128, B], F32)
            nc.vector.tensor_tensor(ag[:], g[:], rstd[:], mybir.AluOpType.mult)
            coefA = constp.tile([128, B], F32)
            nc.sync.dma_start(coefA[0:C, :], ag[C:128, :])
            nc.sync.dma_start(coefA[C:128, :], ag[C:128, :])
            bg = constp.tile([128, B], F32)  # valid on partitions 0:64
            amc = constp.tile([128, B], F32)
            nc.vector.tensor_tensor(amc[:], mean_c[:], coefA[:],
                                    mybir.AluOpType.mult)
            nc.vector.tensor_sub(bg[:], g[:], amc[:])
            coefB = constp.tile([128, B], F32)
            nc.sync.dma_start(coefB[0:C, :], bg[0:C, :])
            nc.sync.dma_start(coefB[C:128, :], bg[0:C, :])

            if BISECT == "nopass2":
                nc.sync.dma_start(outb.ap()[0:C, 0:B], coefA[0:C, :])
                nc.compile()
                return nc

            # ---------------- pass 2 ----------------
            d0, d1_ = P2_DVE, P2_DVE + P2_ACT
            with tc.tile_pool(name="p2o", bufs=3) as p2o:
                for s in range(B):
                    base = s * cs
                    ot = p2o.tile([128, cs], BF16, tag="ot")
                    sa = coefA[:, s : s + 1]
                    sb = coefB[:, s : s + 1]
                    nc.vector.tensor_scalar(
                        ot[:, 0:d0], xc[:, base : base + d0], sa, sb,
                        mybir.AluOpType.mult, mybir.AluOpType.add,
                    )
                    nc.scalar.activation(
                        ot[:, d0:d1_], xc[:, base + d0 : base + d1_], ID,
                        bias=sb, scale=sa,
                    )
                    nc.gpsimd.tensor_scalar(
                        ot[:, d1_:cs], xc[:, base + d1_ : base + cs], sa, sb,
                        mybir.AluOpType.mult, mybir.AluOpType.add,
                    )
                    nc.sync.dma_start(outb.ap()[:, base : base + cs], ot[:])

    nc.compile()
    return nc


_NC_CACHE = {}


def _get_nc(cap=CAP, scap=SCAP, n_cores=N_CORES):
    key = (cap, scap, n_cores)
    if key not in _NC_CACHE:
        _NC_CACHE[key] = build_nc(cap, scap, n_cores)
    return _NC_CACHE[key]


def _deal(idx: np.ndarray, cap: int, n_cores: int):
    """Deal each segment's rows evenly across cores into cap-sized buckets.

    Returns G[(core, seg, cap)] int64 row ids, with N (== len(idx)) marking
    pad slots, and the exact per-segment counts.
    """
    n = len(idx)
    order = np.argsort(idx, kind="stable")
    counts = np.bincount(idx, minlength=B)[:B]
    G = np.full((n_cores, B, cap), n, dtype=np.int64)
    off = 0
    for s in range(B):
        rows_s = order[off : off + counts[s]]
        off += counts[s]
        splits = (np.arange(n_cores + 1) * counts[s]) // n_cores
        for k in range(n_cores):
            ck = rows_s[splits[k] : splits[k + 1]]
            G[k, s, : len(ck)] = ck
    return G, counts


def _to_device_layout(feats: np.ndarray, G: np.ndarray, cap: int):
    """(N, 64) f32 + bucket map -> per-core [128, B*cap//2] f32 arrays."""
    n = feats.shape[0]
    fz = np.concatenate([feats, np.zeros((1, C), np.float32)], axis=0)
    res = []
    for k in range(G.shape[0]):
        Xk = fz[G[k].reshape(-1)]                      # (B*cap, 64)
        B4 = Xk.reshape(B, 2, cap // 2, C)             # (s, h, r, c)
        res.append(
            np.ascontiguousarray(
                B4.transpose(1, 3, 0, 2).reshape(128, B * (cap // 2))
            )
        )
    return res


def _host_inputs(content_feats, style_feats, content_batch_indices,
                 style_batch_indices, cap, scap):
    cfe = np.asarray(content_feats, np.float32)
    sfe = np.asarray(style_feats, np.float32)
    cidx = np.asarray(content_batch_indices, np.int64)
    sidx = np.asarray(style_batch_indices, np.int64)

    Gc, cnt_c = _deal(cidx, cap, N_CORES)
    Gs, cnt_s = _deal(sidx, scap, N_CORES)
    xins = _to_device_layout(cfe, Gc, cap)
    sins = _to_device_layout(sfe, Gs, scap)

    nc_ = np.maximum(cnt_c.astype(np.float64), 1.0)
    ns_ = np.maximum(cnt_s.astype(np.float64), 1.0)
    hrow = np.concatenate(
        [1.0 / nc_, nc_ / np.maximum(nc_ - 1.0, 1.0),
         1.0 / ns_, ns_ / np.maximum(ns_ - 1.0, 1.0)]
    ).astype(np.float32)
    hc = np.ascontiguousarray(np.tile(hrow[None, :], (128, 1)))
    p = np.arange(128)
    pairp = (p[:, None] % C == np.arange(128)[None, :] % C).astype(np.float32)
    mka = np.zeros((128, 2), np.float32)
    mka[0:C, 0] = 1.0
    mka[C:128, 1] = 1.0

    in_maps = [
        {"xin": xins[k], "sin": sins[k], "hc": hc, "pairp": pairp, "mka": mka}
        for k in range(N_CORES)
    ]
    return in_maps, Gc


def _assemble_output(results, Gc, cap, n_rows):
    out = np.zeros((n_rows, C), np.float32)
    for k in range(N_CORES):
        O = np.asarray(results[k]["outb"]).astype(np.float32)
        rows = (
            O.reshape(2, C, B, cap // 2)
            .transpose(2, 0, 3, 1)
            .reshape(B * cap, C)
        )
        gk = Gc[k].reshape(-1)
        mask = gk < n_rows
        out[gk[mask]] = rows[mask]
    return out


def _pick_caps(content_batch_indices, style_batch_indices):
    cidx = np.asarray(content_batch_indices, np.int64)
    sidx = np.asarray(style_batch_indices, np.int64)
    need_c = int(np.ceil(np.bincount(cidx, minlength=B)[:B].max() / N_CORES))
    need_s = int(np.ceil(np.bincount(sidx, minlength=B)[:B].max() / N_CORES))

    def rnd(x):
        return ((x + 63) // 64) * 64

    return max(CAP, rnd(need_c)), max(SCAP, rnd(need_s))


def kernel(
    content_feats: np.ndarray,
    style_feats: np.ndarray,
    content_batch_indices: np.ndarray,
    style_batch_indices: np.ndarray,
    num_batches=B,
) -> np.ndarray:
    n_c = content_feats.shape[0]
    cap, scap = _pick_caps(content_batch_indices, style_batch_indices)
    in_maps, Gc = _host_inputs(
        content_feats, style_feats, content_batch_indices,
        style_batch_indices, cap, scap,
    )
    nc = _get_nc(cap, scap)
    res = bass_utils.run_bass_kernel_spmd(nc, in_maps, core_ids=list(range(N_CORES)))
    return _assemble_output(res.results, Gc, cap, n_c)


# revision 15
# speedup vs baseline: 1.6086x; 1.0376x over previous
"""AdaIN (segment mean/std + EMA of style stats) distributed over 8 TRN2 NeuronCores.

v3 strategy — host-side segment bucketing + transposed layout:
  - The host deals each segment's rows evenly across the 8 cores into
    fixed-capacity buckets (content: CAP rows per (core, segment); style:
    SCAP), padding with zero rows. Pad rows contribute nothing to sums, and
    exact per-segment counts are computed on the host.
  - Data ships TRANSPOSED: partition p = half*64 + channel, free dim = rows.
    Each segment occupies a fixed, compile-time-constant column range, so the
    SPMD instruction stream is static.
  - pass 1 (per core): per segment, DMA the f32 tile, cast to a persistent
    bf16 SBUF cache (fused with accum_out => per-channel sum(x)), and an ACT
    Square pass with accum_out => sum(x^2). Style is reduced straight from
    the staged f32 tiles (no cache).
  - one 32KB AllReduce combines per-core partial sums (the module's own
    dist behavior); a tiny f32 matmul folds the two row-halves together.
  - stats math replicated on every core in [channel, segment] orientation:
    mean/std via host-provided 1/n and n/(n-1), the EMA across batch ids as a
    single tensor_tensor_scan along the segment axis, then a = g_std/c_std,
    b = g_mean - c_mean*a per (channel, segment).
  - pass 2 (per core): per segment, out = x*a + b where a,b are per-partition
    scalars -- one fused op per engine-span on DVE (tensor_scalar), ACT
    (activation Identity w/ scale+bias) and GPSIMD (tensor_scalar), from the
    bf16 cache; bf16 out DMA'd to HBM. Host undoes the permutation.
"""

import os
import sys

import numpy as np

for _p in ("/opt/trn_rl_repo",):
    if _p not in sys.path and os.path.isdir(_p):
        sys.path.insert(0, _p)

from concourse import bacc, bass, bass_utils, mybir, tile

F32 = mybir.dt.float32
BF16 = mybir.dt.bfloat16

N_CORES = 8
C = 64
B = 16
ALPHA = 0.1
EPS = 1e-8

# per-(core, segment) bucket capacities (rows; must be even)
CAP = 7936    # content: expected ~7813 +- 30 after even dealing
SCAP = 1984   # style:   expected ~1953 +- 15

# pass-2 engine column split of each CAP//2-wide segment tile
P2_DVE = 1600
P2_ACT = 1600

BISECT = "full"


def _chunks(total: int, step: int):
    t0 = 0
    while t0 < total:
        yield t0, min(step, total - t0)
        t0 += step


def build_nc(cap: int = CAP, scap: int = SCAP, n_cores: int = N_CORES):
    cs = cap // 2     # content cols per segment
    ss = scap // 2    # style cols per segment
    CC = B * cs       # content cols per core
    SC = B * ss

    nc = bacc.Bacc(
        "TRN2", target_bir_lowering=False, debug=False, num_devices=n_cores
    )
    xin = nc.dram_tensor("xin", [128, CC], F32, kind="ExternalInput")
    sin = nc.dram_tensor("sin", [128, SC], F32, kind="ExternalInput")
    hc = nc.dram_tensor("hc", [128, 4 * B], F32, kind="ExternalInput")
    pairp = nc.dram_tensor("pairp", [128, 128], F32, kind="ExternalInput")
    mask_a = nc.dram_tensor("mka", [128, 2], F32, kind="ExternalInput")
    outb = nc.dram_tensor("outb", [128, CC], BF16, kind="ExternalOutput")

    ID = mybir.ActivationFunctionType.Identity
    SQ = mybir.ActivationFunctionType.Square

    with tile.TileContext(nc) as tc:
        with (
            tc.tile_pool(name="const", bufs=1) as constp,
            tc.tile_pool(name="dram", bufs=1, space="DRAM") as dramp,
        ):
            hc_sb = constp.tile([128, 4 * B], F32)
            nc.sync.dma_start(hc_sb[:], hc.ap())
            pairp_sb = constp.tile([128, 128], F32)
            nc.sync.dma_start(pairp_sb[:], pairp.ap())
            mka = constp.tile([128, 2], F32)   # col0: 1 on 0:64; col1: 1 on 64:128
            nc.sync.dma_start(mka[:], mask_a.ap())

            xc = constp.tile([128, CC], BF16)        # content cache
            Rv = constp.tile([128, 2 * B], F32)      # DVE accums [sx_c|sx_s]
            Ra = constp.tile([128, 2 * B], F32)      # ACT accums [sx2_c|sx2_s]
            Rg = constp.tile([128, 4 * B], F32)      # after AllReduce

            def all_reduce(dst_sb, srcs, tag):
                inb = dramp.tile([128, 2 * B], F32, tag=f"arin_{tag}")
                outd = dramp.tile([128, 2 * B], F32, tag=f"arout_{tag}")
                for j, src in enumerate(srcs):
                    nc.sync.dma_start(inb[:, j * B : (j + 1) * B], src)
                if BISECT == "nocoll":
                    nc.sync.dma_start(outd[:], inb[:])
                else:
                    nc.gpsimd.collective_compute(
                        "AllReduce",
                        mybir.AluOpType.add,
                        replica_groups=[list(range(n_cores))],
                        ins=[inb.opt()],
                        outs=[outd.opt()],
                    )
                nc.sync.dma_start(dst_sb, outd[:])

            # ---------------- pass 1 (style interleaved into content) ------
            Rg_s = constp.tile([128, 2 * B], F32)
            with (
                tc.tile_pool(name="p1s", bufs=2) as p1s,
                tc.tile_pool(name="p1c", bufs=3) as p1c,
                tc.tile_pool(name="p1d", bufs=2) as p1d,
            ):
                def style_seg(s):
                    st = p1s.tile([128, ss], F32, tag="st")
                    nc.sync.dma_start(st[:], sin.ap()[:, s * ss : (s + 1) * ss])
                    d1 = p1d.tile([128, ss], BF16, tag="sd1")
                    nc.scalar.activation(
                        d1[:], st[:], SQ,
                        accum_out=Ra[:, B + s : B + s + 1],
                    )
                    d2 = p1d.tile([128, ss], BF16, tag="sd2")
                    nc.vector.tensor_scalar(
                        d2[:], st[:], 1.0, 0.0, mybir.AluOpType.mult,
                        mybir.AluOpType.add,
                        accum_out=Rv[:, B + s : B + s + 1],
                    )

                def content_seg(s):
                    ct = p1c.tile([128, cs], F32, tag="ct")
                    h = cs // 2
                    nc.sync.dma_start(ct[:, 0:h], xin.ap()[:, s * cs : s * cs + h])
                    nc.sync.dma_start(
                        ct[:, h:cs], xin.ap()[:, s * cs + h : (s + 1) * cs]
                    )
                    nc.vector.tensor_scalar(
                        xc[:, s * cs : (s + 1) * cs], ct[:], 1.0, 0.0,
                        mybir.AluOpType.mult, mybir.AluOpType.add,
                        accum_out=Rv[:, s : s + 1],
                    )
                    d3 = p1d.tile([128, cs], BF16, tag="cd")
                    nc.scalar.activation(
                        d3[:], xc[:, s * cs : (s + 1) * cs], SQ,
                        accum_out=Ra[:, s : s + 1],
                    )

                for s in range(8):
                    content_seg(s)
                    style_seg(2 * s)
                    style_seg(2 * s + 1)
                # style partial sums reduce early: absorbs inter-core launch
                # skew and hides the transfer under the content tail
                all_reduce(Rg_s[:], [Rv[:, B : 2 * B], Ra[:, B : 2 * B]], "s")
                for s in range(8, B):
                    content_seg(s)

            # ---------------- content AllReduce ----------------
            Rg_c = constp.tile([128, 2 * B], F32)
            all_reduce(Rg_c[:], [Rv[:, 0:B], Ra[:, 0:B]], "c")

            # ------------- stats math ([channel, segment], replicated) ------
            # (the style chain below is issued first so it executes inside
            # the content-AllReduce wait window)
            rn_c, fac_c = hc_sb[:, 0:B], hc_sb[:, B : 2 * B]
            rn_s, fac_s = hc_sb[:, 2 * B : 3 * B], hc_sb[:, 3 * B : 4 * B]

            def fold(dst, src):
                # dst[p, j] = src[p%64, j] + src[64 + p%64, j]
                with tc.tile_pool(name="ps_fold", bufs=1, space="PSUM") as psf:
                    ps = psf.tile([128, 2 * B], F32, tag="fold")
                    nc.tensor.matmul(ps[:], pairp_sb[:], src, start=True,
                                     stop=True)
                    nc.vector.tensor_copy(dst, ps[:])

            def seg_stats(S2, rn, fac, mean_out, std_out):
                nc.vector.tensor_tensor(mean_out, S2[:, 0:B], rn,
                                        mybir.AluOpType.mult)
                ex2 = constp.tile([128, B], F32, tag="ts_ex2")
                nc.vector.tensor_tensor(ex2[:], S2[:, B : 2 * B], rn,
                                        mybir.AluOpType.mult)
                m2 = constp.tile([128, B], F32, tag="ts_m2")
                nc.scalar.square(m2[:], mean_out)
                var = constp.tile([128, B], F32, tag="ts_var")
                nc.vector.tensor_sub(var[:], ex2[:], m2[:])
                nc.vector.tensor_tensor(var[:], var[:], fac, mybir.AluOpType.mult)
                nc.vector.tensor_scalar_max(var[:], var[:], 0.0)
                nc.scalar.sqrt(std_out, var[:])
                nc.vector.tensor_scalar_add(std_out, std_out, EPS)

            S_s = constp.tile([128, 2 * B], F32)
            fold(S_s[:], Rg_s[:])
            mean_s = constp.tile([128, B], F32)
            std_s = constp.tile([128, B], F32)
            seg_stats(S_s, rn_s, fac_s, mean_s[:], std_s[:])

            # EMA along segments as one scan: g_j = 0.9*g_{j-1} + w_j*s_j,
            # w_0 = 1 (globals start as batch 0's style stats), w_j = 0.1.
            # Scan input: mean_s on partitions 0:64, std_s on 64:128 (both are
            # replicated, so mask-merge instead of partition moves).
            tmp_sb = constp.tile([128, B], F32)
            nc.vector.tensor_scalar_mul(tmp_sb[:], std_s[:], mka[:, 1:2])
            smw = constp.tile([128, B], F32)
            nc.vector.scalar_tensor_tensor(
                smw[:], mean_s[:], mka[:, 0:1], tmp_sb[:],
                mybir.AluOpType.mult, mybir.AluOpType.add,
            )
            # scale by ALPHA except column 0
            smk = constp.tile([128, B], F32)
            nc.vector.tensor_scalar_mul(smk[:], smw[:], ALPHA)
            nc.vector.tensor_copy(smk[:, 0:1], smw[:, 0:1])
            c09 = constp.tile([128, B], F32)
            nc.vector.memset(c09[:], 1.0 - ALPHA)
            g = constp.tile([128, B], F32)
            nc.vector.tensor_tensor_scan(
                g[:], c09[:], smk[:], 0.0,
                mybir.AluOpType.mult, mybir.AluOpType.add,
            )

            S_c = constp.tile([128, 2 * B], F32)
            fold(S_c[:], Rg_c[:])
            mean_c = constp.tile([128, B], F32)
            std_c = constp.tile([128, B], F32)
            seg_stats(S_c, rn_c, fac_c, mean_c[:], std_c[:])

            # a = g_std/std_c (valid on partitions 64:128);
            # replicate onto both halves, then b = g_mean - mean_c*a.
            rstd = constp.tile([128, B], F32)
            nc.vector.reciprocal(rstd[:], std_c[:])
            ag = constp.tile([128, B], F32)
            nc.vector.tensor_tensor(ag[:], g[:], rstd[:], mybir.AluOpType.mult)
            coefA = constp.tile([128, B], F32)
            nc.sync.dma_start(coefA[0:C, :], ag[C:128, :])
            nc.sync.dma_start(coefA[C:128, :], ag[C:128, :])
            bg = constp.tile([128, B], F32)  # valid on partitions 0:64
            amc = constp.tile([128, B], F32)
            nc.vector.tensor_tensor(amc[:], mean_c[:], coefA[:],
                                    mybir.AluOpType.mult)
            nc.vector.tensor_sub(bg[:], g[:], amc[:])
            coefB = constp.tile([128, B], F32)
            nc.sync.dma_start(coefB[0:C, :], bg[0:C, :])
            nc.sync.dma_start(coefB[C:128, :], bg[0:C, :])

            if BISECT == "nopass2":
                nc.sync.dma_start(outb.ap()[0:C, 0:B], coefA[0:C, :])
                nc.compile()
                return nc

            # ---------------- pass 2 ----------------
            d0, d1_ = P2_DVE, P2_DVE + P2_ACT
            with tc.tile_pool(name="p2o", bufs=3) as p2o:
                for s in range(B):
                    base = s * cs
                    ot = p2o.tile([128, cs], BF16, tag="ot")
                    sa = coefA[:, s : s + 1]
                    sb = coefB[:, s : s + 1]
                    nc.vector.tensor_scalar(
                        ot[:, 0:d0], xc[:, base : base + d0], sa, sb,
                        mybir.AluOpType.mult, mybir.AluOpType.add,
                    )
                    nc.scalar.activation(
                        ot[:, d0:d1_], xc[:, base + d0 : base + d1_], ID,
                        bias=sb, scale=sa,
                    )
                    nc.gpsimd.tensor_scalar(
                        ot[:, d1_:cs], xc[:, base + d1_ : base + cs], sa, sb,
                        mybir.AluOpType.mult, mybir.AluOpType.add,
                    )
                    nc.sync.dma_start(outb.ap()[:, base : base + cs], ot[:])

    nc.compile()
    return nc


_NC_CACHE = {}


def _get_nc(cap=CAP, scap=SCAP, n_cores=N_CORES):
    key = (cap, scap, n_cores)
    if key not in _NC_CACHE:
        _NC_CACHE[key] = build_nc(cap, scap, n_cores)
    return _NC_CACHE[key]


def _deal(idx: np.ndarray, cap: int, n_cores: int):
    """Deal each segment's rows evenly across cores into cap-sized buckets.

    Returns G[(core, seg, cap)] int64 row ids, with N (== len(idx)) marking
    pad slots, and the exact per-segment counts.
    """
    n = len(idx)
    order = np.argsort(idx, kind="stable")
    counts = np.bincount(idx, minlength=B)[:B]
    G = np.full((n_cores, B, cap), n, dtype=np.int64)
    off = 0
    for s in range(B):
        rows_s = order[off : off + counts[s]]
        off += counts[s]
        splits = (np.arange(n_cores + 1) * counts[s]) // n_cores
        for k in range(n_cores):
            ck = rows_s[splits[k] : splits[k + 1]]
            G[k, s, : len(ck)] = ck
    return G, counts


def _to_device_layout(feats: np.ndarray, G: np.ndarray, cap: int):
    """(N, 64) f32 + bucket map -> per-core [128, B*cap//2] f32 arrays."""
    n = feats.shape[0]
    fz = np.concatenate([feats, np.zeros((1, C), np.float32)], axis=0)
    res = []
    for k in range(G.shape[0]):
        Xk = fz[G[k].reshape(-1)]                      # (B*cap, 64)
        B4 = Xk.reshape(B, 2, cap // 2, C)             # (s, h, r, c)
        res.append(
            np.ascontiguousarray(
                B4.transpose(1, 3, 0, 2).reshape(128, B * (cap // 2))
            )
        )
    return res


def _host_inputs(content_feats, style_feats, content_batch_indices,
                 style_batch_indices, cap, scap):
    cfe = np.asarray(content_feats, np.float32)
    sfe = np.asarray(style_feats, np.float32)
    cidx = np.asarray(content_batch_indices, np.int64)
    sidx = np.asarray(style_batch_indices, np.int64)

    Gc, cnt_c = _deal(cidx, cap, N_CORES)
    Gs, cnt_s = _deal(sidx, scap, N_CORES)
    xins = _to_device_layout(cfe, Gc, cap)
    sins = _to_device_layout(sfe, Gs, scap)

    nc_ = np.maximum(cnt_c.astype(np.float64), 1.0)
    ns_ = np.maximum(cnt_s.astype(np.float64), 1.0)
    hrow = np.concatenate(
        [1.0 / nc_, nc_ / np.maximum(nc_ - 1.0, 1.0),
         1.0 / ns_, ns_ / np.maximum(ns_ - 1.0, 1.0)]
    ).astype(np.float32)
    hc = np.ascontiguousarray(np.tile(hrow[None, :], (128, 1)))
    p = np.arange(128)
    pairp = (p[:, None] % C == np.arange(128)[None, :] % C).astype(np.float32)
    mka = np.zeros((128, 2), np.float32)
    mka[0:C, 0] = 1.0
    mka[C:128, 1] = 1.0

    in_maps = [
        {"xin": xins[k], "sin": sins[k], "hc": hc, "pairp": pairp, "mka": mka}
        for k in range(N_CORES)
    ]
    return in_maps, Gc


def _assemble_output(results, Gc, cap, n_rows):
    out = np.zeros((n_rows, C), np.float32)
    for k in range(N_CORES):
        O = np.asarray(results[k]["outb"]).astype(np.float32)
        rows = (
            O.reshape(2, C, B, cap // 2)
            .transpose(2, 0, 3, 1)
            .reshape(B * cap, C)
        )
        gk = Gc[k].reshape(-1)
        mask = gk < n_rows
        out[gk[mask]] = rows[mask]
    return out


def _pick_caps(content_batch_indices, style_batch_indices):
    cidx = np.asarray(content_batch_indices, np.int64)
    sidx = np.asarray(style_batch_indices, np.int64)
    need_c = int(np.ceil(np.bincount(cidx, minlength=B)[:B].max() / N_CORES))
    need_s = int(np.ceil(np.bincount(sidx, minlength=B)[:B].max() / N_CORES))

    def rnd(x):
        return ((x + 63) // 64) * 64

    return max(CAP, rnd(need_c)), max(SCAP, rnd(need_s))


def kernel(
    content_feats: np.ndarray,
    style_feats: np.ndarray,
    content_batch_indices: np.ndarray,
    style_batch_indices: np.ndarray,
    num_batches=B,
) -> np.ndarray:
    n_c = content_feats.shape[0]
    cap, scap = _pick_caps(content_batch_indices, style_batch_indices)
    in_maps, Gc = _host_inputs(
        content_feats, style_feats, content_batch_indices,
        style_batch_indices, cap, scap,
    )
    nc = _get_nc(cap, scap)
    res = bass_utils.run_bass_kernel_spmd(nc, in_maps, core_ids=list(range(N_CORES)))
    return _assemble_output(res.results, Gc, cap, n_c)
